# revision 2
# baseline (speedup 1.0000x reference)
"""ColumnParallelLinear + per-token LoRA (punica add_lora) on 8 NeuronCores.

out = x @ W^T + b + B[idx] @ (A[idx] @ x^T), idx==-1 skips LoRA.

Sharding: tensor-parallel over the output dim (vLLM ColumnParallelLinear):
weight, bias and lora_b are sharded 512-wide per core; lora_a and indices
are replicated. The per-token LoRA shrink (s = A @ x) is sharded over
tokens (256/core) and shared via an on-chip AllGather in fp8; the LoRA
expand is folded into the main accumulation as a dense matmul against the
routing-masked shrink (s_masked[t, (l,r)] = (idx[t]==l) * s[t, (l,r)]).

Perf notes (measured on HW):
- The PE streams moving columns at ~1.95 GHz regardless of dtype, so the
  base matmul stays bf16 (fp8 for the base fails the 2e-2 gate: measured
  3.9e-2). The LoRA shrink/expand run as fp8e4 DoubleRow matmuls (2 K-tiles
  per instruction), halving their PE occupancy; their quantization error
  only feeds the small LoRA correction term (total rel err ~6e-3).
- The base matmul is tiled into 8 token groups of 256 and interleaved with
  the shrink chunks at the start so the PE never waits on the big x/w DMA
  streams; shrink inputs ride the Activation-engine DMA queue while
  x/w ride the SP queue.
- Expand groups trail base groups by 2 so the fp8 AllGather hides under
  ~34us of base matmuls; output is stored bf16 (upcast on host).
"""
import json

import numpy as np
import ml_dtypes

import concourse.bass as bass
import concourse.mybir as mybir
import concourse.tile as tile
from concourse.bass_utils import run_bass_kernel_spmd

T, H, O, L, R = 2048, 4096, 4096, 32, 16
N_CORES = 8
O_SH = O // N_CORES          # 512  output cols per core
T_LOC = T // N_CORES         # 256  tokens whose LoRA-shrink this core computes
KB = H // 128                # 32   contraction blocks
LR = L * R                   # 512  stacked (lora, rank) rows
G = 8                        # base-matmul token groups
TG = T // G                  # 256  tokens per group
BF16 = mybir.dt.bfloat16
F32 = mybir.dt.float32
F8 = mybir.dt.float8e4
DR = mybir.MatmulPerfMode.DoubleRow
A_SCALE = 16.0               # lora_a is pre-scaled x16 into fp8
B_SCALE = 64.0               # lora_b is pre-scaled x64 into fp8
D_SCALE = 1.0 / (A_SCALE * B_SCALE)   # undo both during the final combine


def _split_waits(raw: bytes) -> bytes:
    """This walrus build rejects instructions carrying more than one sync
    wait ("Too many sync wait commands"), but Tile attaches one wait per
    producing proc. Hoist all but one wait of each instruction onto
    single-wait NoOps inserted just before it on the same engine — the
    engine executes its stream in order, so the gating is identical."""
    m = json.loads(raw)
    ctr = 0
    for f in m["functions"]:
        for b in f["blocks"]:
            out = []
            for inst in b["instructions"]:
                si = inst.get("sync_info")
                waits = si.get("on_wait") if si else None
                if waits and len(waits) > 1:
                    for w in waits[:-1]:
                        ctr += 1
                        out.append({
                            "debug": inst.get("debug", 0),
                            "engine": inst["engine"],
                            "ins": [],
                            "name": f"I-wsplit-{ctr}",
                            "opcode": "NoOp",
                            "outs": [],
                            "sync_info": {"on_update": [], "on_wait": [w]},
                        })
                    si["on_wait"] = [waits[-1]]
                out.append(inst)
            b["instructions"] = out
    return json.dumps(m).encode()


class _WaitSplitBass(bass.Bass):
    def to_json_bytes(self) -> bytes:
        return _split_waits(super().to_json_bytes())


def _build() -> bass.Bass:
    nc = _WaitSplitBass()
    # all streamed inputs are PE-tile-major: [128 h-partitions, ..., free]
    xG = nc.dram_tensor("xG", [128, G, KB, TG], BF16, kind="ExternalInput")
    xl_r = nc.dram_tensor("xl_r", [128, KB, T_LOC], F8, kind="ExternalInput")
    wTr = nc.dram_tensor("wTr", [128, KB, O_SH], BF16, kind="ExternalInput")
    aTr = nc.dram_tensor("aTr", [128, KB, LR], F8, kind="ExternalInput")
    bTr = nc.dram_tensor("bTr", [128, 4, O_SH], F8, kind="ExternalInput")
    bias_row = nc.dram_tensor("bias_row", [1, O_SH], BF16, kind="ExternalInput")
    idx_bc_d = nc.dram_tensor("idx_bc", [128, T_LOC], F32, kind="ExternalInput")
    lrow_d = nc.dram_tensor("lrow", [128, 4], F32, kind="ExternalInput")
    out = nc.dram_tensor("out", [T, O_SH], BF16, kind="ExternalOutput")

    with tile.TileContext(nc) as tc:
        with (
            tc.tile_pool(name="res", bufs=1) as res,          # long-lived SBUF
            tc.tile_pool(name="stream", bufs=4) as stream,    # streamed SBUF
            tc.tile_pool(name="ps", bufs=2, space="PSUM") as ps,
            tc.tile_pool(name="dram", bufs=1, space="DRAM") as dram,
        ):
            # ------------- shrink-path inputs on the Activation DMA queue ---
            bias_r = res.tile([1, O_SH], BF16, name="bias_r")
            nc.scalar.dma_start(bias_r[:], bias_row[:])
            idx_bc = res.tile([128, T_LOC], F32, name="idx_bc_t")
            nc.scalar.dma_start(idx_bc[:], idx_bc_d[:])
            lrow = res.tile([128, 4], F32, name="lrow_t")
            nc.scalar.dma_start(lrow[:], lrow_d[:])

            xl_all = res.tile([128, KB, T_LOC], F8, name="xl_all")
            at_all = res.tile([128, KB, LR], F8, name="at_all")
            for c in range(4):
                kc = KB // 4
                nc.scalar.dma_start(
                    xl_all[:, c * kc:(c + 1) * kc, :],
                    xl_r[:, c * kc:(c + 1) * kc, :],
                )
                nc.scalar.dma_start(
                    at_all[:, c * kc:(c + 1) * kc, :],
                    aTr[:, c * kc:(c + 1) * kc, :],
                )
            bt_all = res.tile([128, 4, O_SH], F8, name="bt_all")
            nc.gpsimd.dma_start(bt_all[:], bTr[:])

            # ------------- base-path inputs on the SP DMA queue -------------
            wt_all = res.tile([128, KB, O_SH], BF16, name="wt_all")

            # bias broadcast [128, 512] f32 via K=1 ones-matmul
            ones_t = res.tile([1, 128], BF16, name="ones_t")
            nc.vector.memset(ones_t[:], 1.0)
            bias_ps = ps.tile([128, O_SH], F32, name="bias_ps", tag="psd0")
            nc.tensor.matmul(bias_ps[:], ones_t[:], bias_r[:], start=True, stop=True)
            bias_bc = res.tile([128, O_SH], F32, name="bias_bc")
            nc.vector.tensor_copy(bias_bc[:], bias_ps[:])

            # base accumulations land in SBUF (with bias) as each group
            # finishes; the expand is combined during the store.
            base_sb = res.tile([128, 2 * G * O_SH], F32, name="base_sb")

            ps_s = [ps.tile([128, T_LOC], F32, name=f"ps_s{m}", tag=tg)
                    for m, tg in enumerate(["pso0", "pso1", "psd0", "psd1"])]

            def shrink_chunk(c):
                # fp8 DoubleRow: each matmul contracts kb pair (2k2, 2k2+1)
                for k2 in range(4 * c, 4 * (c + 1)):
                    for m in range(4):
                        nc.tensor.matmul(
                            ps_s[m][:],
                            at_all[:, 2 * k2:2 * k2 + 2, m * 128:(m + 1) * 128],
                            xl_all[:, 2 * k2:2 * k2 + 2, :],
                            start=(k2 == 0),
                            stop=(k2 == 15),
                            perf_mode=DR,
                        )

            def base_group(g, ps_o):
                xs = stream.tile([128, KB, TG], BF16, name="xs", tag="xs", bufs=3)
                if g == 0:
                    # pair wt chunks with the points of the kb sweep that
                    # first need them
                    nc.sync.dma_start(xs[:, 0:16, :], xG[:, g:g + 1, 0:16, :])
                    nc.sync.dma_start(wt_all[:, 0:8, :], wTr[:, 0:8, :])
                    nc.sync.dma_start(wt_all[:, 8:16, :], wTr[:, 8:16, :])
                    nc.sync.dma_start(xs[:, 16:32, :], xG[:, g:g + 1, 16:32, :])
                    nc.sync.dma_start(wt_all[:, 16:24, :], wTr[:, 16:24, :])
                    nc.sync.dma_start(wt_all[:, 24:32, :], wTr[:, 24:32, :])
                else:
                    nc.sync.dma_start(xs[:], xG[:, g:g + 1, :, :])
                for kb in range(KB):
                    for tt in range(2):
                        nc.tensor.matmul(
                            ps_o[tt][:],
                            xs[:, kb, tt * 128:(tt + 1) * 128],
                            wt_all[:, kb, :],
                            start=(kb == 0),
                            stop=(kb == KB - 1),
                        )
                for tt in range(2):
                    nc.vector.tensor_tensor(
                        base_sb[:, (2 * g + tt) * O_SH:(2 * g + tt + 1) * O_SH],
                        ps_o[tt][:],
                        bias_bc[:],
                        op=mybir.AluOpType.add,
                    )

            # preamble: interleave base group 0 with the shrink so the PE
            # tracks the DMA streams instead of waiting on either
            ps_o0 = [ps.tile([128, O_SH], F32, name=f"ps_o0_{t}", tag=f"pso{t}")
                     for t in range(2)]
            xs0 = stream.tile([128, KB, TG], BF16, name="xs", tag="xs", bufs=3)
            nc.sync.dma_start(xs0[:, 0:16, :], xG[:, 0:1, 0:16, :])
            nc.sync.dma_start(wt_all[:, 0:8, :], wTr[:, 0:8, :])
            nc.sync.dma_start(wt_all[:, 8:16, :], wTr[:, 8:16, :])
            nc.sync.dma_start(xs0[:, 16:32, :], xG[:, 0:1, 16:32, :])
            nc.sync.dma_start(wt_all[:, 16:24, :], wTr[:, 16:24, :])
            nc.sync.dma_start(wt_all[:, 24:32, :], wTr[:, 24:32, :])

            for c in range(4):
                shrink_chunk(c)
                for kb in range(8 * c, 8 * (c + 1)):
                    for tt in range(2):
                        nc.tensor.matmul(
                            ps_o0[tt][:],
                            xs0[:, kb, tt * 128:(tt + 1) * 128],
                            wt_all[:, kb, :],
                            start=(kb == 0),
                            stop=(kb == KB - 1),
                        )
            for tt in range(2):
                nc.vector.tensor_tensor(
                    base_sb[:, tt * O_SH:(tt + 1) * O_SH],
                    ps_o0[tt][:],
                    bias_bc[:],
                    op=mybir.AluOpType.add,
                )

            # routing mask + rescale + fp8 downcast, fused:
            # sm = (idx==l(p)) * (A_SCALE * s)
            sm = res.tile([128, 4 * T_LOC], F8, name="sm")
            for m in range(4):
                nc.vector.scalar_tensor_tensor(
                    sm[:, m * T_LOC:(m + 1) * T_LOC],
                    idx_bc[:],
                    lrow[:, m:m + 1],
                    ps_s[m][:],
                    op0=mybir.AluOpType.is_equal,
                    op1=mybir.AluOpType.mult,
                )
            cc_in = dram.tile([LR, T_LOC], F8, name="cc_in")
            nc.gpsimd.dma_start(
                cc_in[:].rearrange("(m p) t -> p m t", p=128),
                sm[:].rearrange("p (m t) -> p m t", t=T_LOC),
            )
            cc_out = dram.tile([N_CORES, LR, T_LOC], F8, name="cc_out",
                               addr_space="Shared")
            nc.gpsimd.collective_compute(
                "AllGather",
                mybir.AluOpType.bypass,
                replica_groups=[list(range(N_CORES))],
                ins=[cc_in.opt()],
                outs=[cc_out.opt()],
            )

            def tail(g):
                ps_d = [
                    ps.tile([128, O_SH], F32, name=f"ps_d{g}_{t}", tag=f"psd{t}")
                    for t in range(2)
                ]
                st = stream.tile([128, 4, TG], F8, name="st", tag="st", bufs=3)
                for blk in range(4):
                    nc.gpsimd.dma_start(
                        st[:, blk:blk + 1, :],
                        cc_out[g, blk * 128:(blk + 1) * 128, :],
                    )
                for tt in range(2):
                    for d2 in range(2):
                        nc.tensor.matmul(
                            ps_d[tt][:],
                            st[:, 2 * d2:2 * d2 + 2, tt * 128:(tt + 1) * 128],
                            bt_all[:, 2 * d2:2 * d2 + 2, :],
                            start=(d2 == 0),
                            stop=(d2 == 1),
                            perf_mode=DR,
                        )
                ot = stream.tile([128, 2 * O_SH], BF16, name="ot", tag="ot", bufs=2)
                for tt in range(2):
                    nc.vector.scalar_tensor_tensor(
                        ot[:, tt * O_SH:(tt + 1) * O_SH],
                        ps_d[tt][:],
                        D_SCALE,
                        base_sb[:, (2 * g + tt) * O_SH:(2 * g + tt + 1) * O_SH],
                        op0=mybir.AluOpType.mult,
                        op1=mybir.AluOpType.add,
                    )
                dst = out[g * TG:(g + 1) * TG, :].rearrange(
                    "(tt p) o -> p tt o", p=128
                )
                nc.sync.dma_start(dst, ot[:].rearrange("p (tt o) -> p tt o", o=O_SH))

            # expand trails base by two groups: the fp8 AllGather gets ~35us
            # of slack before the first expand matmul needs its output
            for g in range(1, G):
                ps_o = [ps.tile([128, O_SH], F32, name=f"ps_o{g}_{t}",
                                tag=f"pso{t}") for t in range(2)]
                base_group(g, ps_o)
                if g >= 2:
                    tail(g - 2)
            tail(G - 2)
            tail(G - 1)
    return nc


_NC_CACHE = None


def build_in_maps(x, weight, bias, lora_a, lora_b, indices):
    bf = ml_dtypes.bfloat16
    f8 = mybir.dt.np(F8)

    # [128 h-partitions, group, kb, token] PE-tile-major layout
    xG = np.ascontiguousarray(
        x.astype(bf).reshape(G, TG, KB, 128).transpose(3, 0, 2, 1))
    aTr = np.ascontiguousarray(
        (lora_a * A_SCALE).astype(f8).reshape(LR, H).T
        .reshape(KB, 128, LR).transpose(1, 0, 2))                   # (128,KB,LR)
    idx_f = indices.astype(np.float32)                              # (T,)
    lrow = np.broadcast_to(
        (np.arange(128)[:, None] // 16).astype(np.float32), (128, 4)
    ).copy()
    lrow = lrow + (np.arange(4)[None, :] * 8).astype(np.float32)    # (128, 4)

    in_maps = []
    for c in range(N_CORES):
        wTc = np.ascontiguousarray(
            weight[c * O_SH:(c + 1) * O_SH, :].astype(bf).T
            .reshape(KB, 128, O_SH).transpose(1, 0, 2))             # (128,KB,O_SH)
        bTc = np.ascontiguousarray(
            (lora_b[:, c * O_SH:(c + 1) * O_SH, :] * B_SCALE).astype(f8)
            .transpose(0, 2, 1).reshape(LR, O_SH)                   # ((l,r), o)
            .reshape(4, 128, O_SH).transpose(1, 0, 2))              # (128,4,O_SH)
        bias_c = np.ascontiguousarray(
            bias[c * O_SH:(c + 1) * O_SH].astype(bf))[None, :]
        idx_bc = np.broadcast_to(
            idx_f[c * T_LOC:(c + 1) * T_LOC][None, :], (128, T_LOC)
        ).copy()
        xl_c = np.ascontiguousarray(
            x[c * T_LOC:(c + 1) * T_LOC, :].astype(f8).T
            .reshape(KB, 128, T_LOC).transpose(1, 0, 2))            # (128,KB,T_LOC)
        in_maps.append({
            "xG": xG, "xl_r": xl_c, "wTr": wTc, "aTr": aTr, "bTr": bTc,
            "bias_row": bias_c, "idx_bc": idx_bc, "lrow": lrow,
        })
    return in_maps


def kernel(x, weight, bias, lora_a, lora_b, indices):
    global _NC_CACHE
    in_maps = build_in_maps(x, weight, bias, lora_a, lora_b, indices)
    if _NC_CACHE is None:
        _NC_CACHE = _build()
    r = run_bass_kernel_spmd(_NC_CACHE, in_maps, core_ids=list(range(N_CORES)))
    return np.concatenate(
        [r.results[c]["out"].astype(np.float32) for c in range(N_CORES)], axis=1)


# revision 6
# speedup vs baseline: 1.0672x; 1.0672x over previous
"""ColumnParallelLinear + per-token LoRA (punica add_lora) on 8 NeuronCores.

out = x @ W^T + b + B[idx] @ (A[idx] @ x^T), idx==-1 skips LoRA.

Sharding: tensor-parallel over the output dim (vLLM ColumnParallelLinear):
weight, bias and lora_b are sharded 512-wide per core; lora_a and indices
are replicated. The per-token LoRA shrink (s = A @ x) is sharded over
tokens (256/core) and shared via an on-chip AllGather in fp8; the LoRA
expand is folded into the main accumulation as a dense matmul against the
routing-masked shrink (s_masked[t, (l,r)] = (idx[t]==l) * s[t, (l,r)]).

Perf notes (measured on HW):
- The PE streams moving columns at ~1.95 GHz regardless of dtype, so the
  base matmul stays bf16 (fp8 for the base fails the 2e-2 gate: measured
  3.9e-2). The LoRA shrink/expand run as fp8e4 DoubleRow matmuls (2 K-tiles
  per instruction), halving their PE occupancy; their quantization error
  only feeds the small LoRA correction term (total rel err ~6e-3).
- The base matmul is tiled into 8 token groups of 256 and interleaved with
  the shrink chunks at the start so the PE never waits on the big x/w DMA
  streams; shrink inputs ride the Activation-engine DMA queue while
  x/w ride the SP queue.
- Expand groups trail base groups by 2 so the fp8 AllGather hides under
  ~34us of base matmuls; output is stored bf16 (upcast on host).
"""
import json

import numpy as np
import ml_dtypes

import concourse.bass as bass
import concourse.mybir as mybir
import concourse.tile as tile
from concourse.bass_utils import run_bass_kernel_spmd

T, H, O, L, R = 2048, 4096, 4096, 32, 16
N_CORES = 8
O_SH = O // N_CORES          # 512  output cols per core
T_LOC = T // N_CORES         # 256  tokens whose LoRA-shrink this core computes
KB = H // 128                # 32   contraction blocks
LR = L * R                   # 512  stacked (lora, rank) rows
G = 8                        # base-matmul token groups
TG = T // G                  # 256  tokens per group
BF16 = mybir.dt.bfloat16
F32 = mybir.dt.float32
F8 = mybir.dt.float8e4
DR = mybir.MatmulPerfMode.DoubleRow
A_SCALE = 16.0               # lora_a is pre-scaled x16 into fp8
B_SCALE = 64.0               # lora_b is pre-scaled x64 into fp8
D_SCALE = 1.0 / (A_SCALE * B_SCALE)   # undo both during the final combine


def _split_waits(raw: bytes) -> bytes:
    """This walrus build rejects instructions carrying more than one sync
    wait ("Too many sync wait commands"), but Tile attaches one wait per
    producing proc. Hoist all but one wait of each instruction onto
    single-wait NoOps inserted just before it on the same engine — the
    engine executes its stream in order, so the gating is identical."""
    m = json.loads(raw)
    ctr = 0
    for f in m["functions"]:
        for b in f["blocks"]:
            out = []
            for inst in b["instructions"]:
                si = inst.get("sync_info")
                waits = si.get("on_wait") if si else None
                if waits and len(waits) > 1:
                    for w in waits[:-1]:
                        ctr += 1
                        out.append({
                            "debug": inst.get("debug", 0),
                            "engine": inst["engine"],
                            "ins": [],
                            "name": f"I-wsplit-{ctr}",
                            "opcode": "NoOp",
                            "outs": [],
                            "sync_info": {"on_update": [], "on_wait": [w]},
                        })
                    si["on_wait"] = [waits[-1]]
                out.append(inst)
            b["instructions"] = out
    return json.dumps(m).encode()


class _WaitSplitBass(bass.Bass):
    def to_json_bytes(self) -> bytes:
        return _split_waits(super().to_json_bytes())


def _build() -> bass.Bass:
    nc = _WaitSplitBass()
    # all streamed inputs are PE-tile-major: [128 h-partitions, ..., free]
    xG = nc.dram_tensor("xG", [128, G, KB, TG], BF16, kind="ExternalInput")
    xl_r = nc.dram_tensor("xl_r", [128, KB, T_LOC], F8, kind="ExternalInput")
    wTr = nc.dram_tensor("wTr", [128, KB, O_SH], BF16, kind="ExternalInput")
    aTr = nc.dram_tensor("aTr", [128, KB, LR], F8, kind="ExternalInput")
    bTr = nc.dram_tensor("bTr", [128, 4, O_SH], F8, kind="ExternalInput")
    bias_row = nc.dram_tensor("bias_row", [1, O_SH], BF16, kind="ExternalInput")
    idx_bc_d = nc.dram_tensor("idx_bc", [128, T_LOC], F32, kind="ExternalInput")
    lrow_d = nc.dram_tensor("lrow", [128, 4], F32, kind="ExternalInput")
    out = nc.dram_tensor("out", [T, O_SH], BF16, kind="ExternalOutput")

    with tile.TileContext(nc) as tc:
        with (
            tc.tile_pool(name="res", bufs=1) as res,          # long-lived SBUF
            tc.tile_pool(name="stream", bufs=4) as stream,    # streamed SBUF
            tc.tile_pool(name="ps", bufs=2, space="PSUM") as ps,
            tc.tile_pool(name="dram", bufs=1, space="DRAM") as dram,
        ):
            # ------------- shrink-path inputs on the Activation DMA queue ---
            bias_r = res.tile([1, O_SH], BF16, name="bias_r")
            nc.scalar.dma_start(bias_r[:], bias_row[:])
            idx_bc = res.tile([128, T_LOC], F32, name="idx_bc_t")
            nc.scalar.dma_start(idx_bc[:], idx_bc_d[:])
            lrow = res.tile([128, 4], F32, name="lrow_t")
            nc.scalar.dma_start(lrow[:], lrow_d[:])

            xl_all = res.tile([128, KB, T_LOC], F8, name="xl_all")
            at_all = res.tile([128, KB, LR], F8, name="at_all")
            # first chunk small so the shrink starts as early as possible
            for lo, hi in ((0, 4), (4, 8), (8, 16), (16, 24), (24, 32)):
                nc.scalar.dma_start(xl_all[:, lo:hi, :], xl_r[:, lo:hi, :])
                nc.scalar.dma_start(at_all[:, lo:hi, :], aTr[:, lo:hi, :])
            bt_all = res.tile([128, 4, O_SH], F8, name="bt_all")
            nc.gpsimd.dma_start(bt_all[:], bTr[:])

            # ------------- base-path inputs on the SP DMA queue -------------
            wt_all = res.tile([128, KB, O_SH], BF16, name="wt_all")

            # bias broadcast [128, 512] f32 via K=1 ones-matmul
            ones_t = res.tile([1, 128], BF16, name="ones_t")
            nc.vector.memset(ones_t[:], 1.0)
            bias_ps = ps.tile([128, O_SH], F32, name="bias_ps", tag="psd0")
            nc.tensor.matmul(bias_ps[:], ones_t[:], bias_r[:], start=True, stop=True)
            bias_bc = res.tile([128, O_SH], F32, name="bias_bc")
            nc.vector.tensor_copy(bias_bc[:], bias_ps[:])

            # base accumulations land in SBUF (with bias) as each group
            # finishes; the expand is combined during the store.
            base_sb = res.tile([128, 2 * G * O_SH], F32, name="base_sb")

            ps_s = [ps.tile([128, T_LOC], F32, name=f"ps_s{m}", tag=tg)
                    for m, tg in enumerate(["pso0", "pso1", "psd0", "psd1"])]

            # LoRA shrink first: fp8 DoubleRow, each matmul contracts the kb
            # pair (2k2, 2k2+1). Runs DMA-paced so cc_in issues ~24us in —
            # the AllGather has a ~30us start lag plus ~23us duration, and
            # this chain is what gates the expand groups.
            for k2 in range(16):
                for m in range(4):
                    nc.tensor.matmul(
                        ps_s[m][:],
                        at_all[:, 2 * k2:2 * k2 + 2, m * 128:(m + 1) * 128],
                        xl_all[:, 2 * k2:2 * k2 + 2, :],
                        start=(k2 == 0),
                        stop=(k2 == 15),
                        perf_mode=DR,
                    )

            def base_group(g, ps_o):
                xs = stream.tile([128, KB, TG], BF16, name="xs", tag="xs", bufs=3)
                if g == 0:
                    # pair wt chunks with the points of the kb sweep that
                    # first need them
                    nc.sync.dma_start(xs[:, 0:16, :], xG[:, g:g + 1, 0:16, :])
                    nc.sync.dma_start(wt_all[:, 0:8, :], wTr[:, 0:8, :])
                    nc.sync.dma_start(wt_all[:, 8:16, :], wTr[:, 8:16, :])
                    nc.sync.dma_start(xs[:, 16:32, :], xG[:, g:g + 1, 16:32, :])
                    nc.sync.dma_start(wt_all[:, 16:24, :], wTr[:, 16:24, :])
                    nc.sync.dma_start(wt_all[:, 24:32, :], wTr[:, 24:32, :])
                else:
                    nc.sync.dma_start(xs[:], xG[:, g:g + 1, :, :])
                for kb in range(KB):
                    for tt in range(2):
                        nc.tensor.matmul(
                            ps_o[tt][:],
                            xs[:, kb, tt * 128:(tt + 1) * 128],
                            wt_all[:, kb, :],
                            start=(kb == 0),
                            stop=(kb == KB - 1),
                        )
                for tt in range(2):
                    nc.vector.tensor_tensor(
                        base_sb[:, (2 * g + tt) * O_SH:(2 * g + tt + 1) * O_SH],
                        ps_o[tt][:],
                        bias_bc[:],
                        op=mybir.AluOpType.add,
                    )

            # routing mask + rescale + fp8 downcast, fused:
            # sm = (idx==l(p)) * (A_SCALE * s)
            sm = res.tile([128, 4 * T_LOC], F8, name="sm")
            for m in range(4):
                nc.vector.scalar_tensor_tensor(
                    sm[:, m * T_LOC:(m + 1) * T_LOC],
                    idx_bc[:],
                    lrow[:, m:m + 1],
                    ps_s[m][:],
                    op0=mybir.AluOpType.is_equal,
                    op1=mybir.AluOpType.mult,
                )
            cc_in = dram.tile([LR, T_LOC], F8, name="cc_in")
            nc.gpsimd.dma_start(
                cc_in[:].rearrange("(m p) t -> p m t", p=128),
                sm[:].rearrange("p (m t) -> p m t", t=T_LOC),
            )
            cc_out = dram.tile([N_CORES, LR, T_LOC], F8, name="cc_out",
                               addr_space="Shared")
            nc.gpsimd.collective_compute(
                "AllGather",
                mybir.AluOpType.bypass,
                replica_groups=[list(range(N_CORES))],
                ins=[cc_in.opt()],
                outs=[cc_out.opt()],
            )

            def tail(g):
                ps_d = [
                    ps.tile([128, O_SH], F32, name=f"ps_d{g}_{t}", tag=f"psd{t}")
                    for t in range(2)
                ]
                st = stream.tile([128, 4, TG], F8, name="st", tag="st", bufs=3)
                for blk in range(4):
                    # scalar queue: idle after the preamble, and unlike the
                    # gpsimd queue it is not blocked behind the collective
                    nc.scalar.dma_start(
                        st[:, blk:blk + 1, :],
                        cc_out[g, blk * 128:(blk + 1) * 128, :],
                    )
                for tt in range(2):
                    for d2 in range(2):
                        nc.tensor.matmul(
                            ps_d[tt][:],
                            st[:, 2 * d2:2 * d2 + 2, tt * 128:(tt + 1) * 128],
                            bt_all[:, 2 * d2:2 * d2 + 2, :],
                            start=(d2 == 0),
                            stop=(d2 == 1),
                            perf_mode=DR,
                        )
                ot = stream.tile([128, 2 * O_SH], BF16, name="ot", tag="ot", bufs=2)
                for tt in range(2):
                    nc.vector.scalar_tensor_tensor(
                        ot[:, tt * O_SH:(tt + 1) * O_SH],
                        ps_d[tt][:],
                        D_SCALE,
                        base_sb[:, (2 * g + tt) * O_SH:(2 * g + tt + 1) * O_SH],
                        op0=mybir.AluOpType.mult,
                        op1=mybir.AluOpType.add,
                    )
                dst = out[g * TG:(g + 1) * TG, :].rearrange(
                    "(tt p) o -> p tt o", p=128
                )
                nc.sync.dma_start(dst, ot[:].rearrange("p (tt o) -> p tt o", o=O_SH))

            # expand trails base by three groups: the fp8 AllGather (done
            # ~80us in) is always ready before the first expand matmul
            for g in range(G):
                ps_o = [ps.tile([128, O_SH], F32, name=f"ps_o{g}_{t}",
                                tag=f"pso{t}") for t in range(2)]
                base_group(g, ps_o)
                if g >= 4:
                    tail(g - 4)
            for g in range(G - 4, G):
                tail(g)
    return nc


_NC_CACHE = None


def build_in_maps(x, weight, bias, lora_a, lora_b, indices):
    bf = ml_dtypes.bfloat16
    f8 = mybir.dt.np(F8)

    # [128 h-partitions, group, kb, token] PE-tile-major layout
    xG = np.ascontiguousarray(
        x.astype(bf).reshape(G, TG, KB, 128).transpose(3, 0, 2, 1))
    aTr = np.ascontiguousarray(
        (lora_a * A_SCALE).astype(f8).reshape(LR, H).T
        .reshape(KB, 128, LR).transpose(1, 0, 2))                   # (128,KB,LR)
    idx_f = indices.astype(np.float32)                              # (T,)
    lrow = np.broadcast_to(
        (np.arange(128)[:, None] // 16).astype(np.float32), (128, 4)
    ).copy()
    lrow = lrow + (np.arange(4)[None, :] * 8).astype(np.float32)    # (128, 4)

    in_maps = []
    for c in range(N_CORES):
        wTc = np.ascontiguousarray(
            weight[c * O_SH:(c + 1) * O_SH, :].astype(bf).T
            .reshape(KB, 128, O_SH).transpose(1, 0, 2))             # (128,KB,O_SH)
        bTc = np.ascontiguousarray(
            (lora_b[:, c * O_SH:(c + 1) * O_SH, :] * B_SCALE).astype(f8)
            .transpose(0, 2, 1).reshape(LR, O_SH)                   # ((l,r), o)
            .reshape(4, 128, O_SH).transpose(1, 0, 2))              # (128,4,O_SH)
        bias_c = np.ascontiguousarray(
            bias[c * O_SH:(c + 1) * O_SH].astype(bf))[None, :]
        idx_bc = np.broadcast_to(
            idx_f[c * T_LOC:(c + 1) * T_LOC][None, :], (128, T_LOC)
        ).copy()
        xl_c = np.ascontiguousarray(
            x[c * T_LOC:(c + 1) * T_LOC, :].astype(f8).T
            .reshape(KB, 128, T_LOC).transpose(1, 0, 2))            # (128,KB,T_LOC)
        in_maps.append({
            "xG": xG, "xl_r": xl_c, "wTr": wTc, "aTr": aTr, "bTr": bTc,
            "bias_row": bias_c, "idx_bc": idx_bc, "lrow": lrow,
        })
    return in_maps


def kernel(x, weight, bias, lora_a, lora_b, indices):
    global _NC_CACHE
    in_maps = build_in_maps(x, weight, bias, lora_a, lora_b, indices)
    if _NC_CACHE is None:
        _NC_CACHE = _build()
    r = run_bass_kernel_spmd(_NC_CACHE, in_maps, core_ids=list(range(N_CORES)))
    return np.concatenate(
        [r.results[c]["out"].astype(np.float32) for c in range(N_CORES)], axis=1)


# revision 9
# speedup vs baseline: 1.0958x; 1.0268x over previous
"""ColumnParallelLinear + per-token LoRA (punica add_lora) on 8 NeuronCores.

out = x @ W^T + b + B[idx] @ (A[idx] @ x^T), idx==-1 skips LoRA.

Sharding: tensor-parallel over the output dim (vLLM ColumnParallelLinear):
weight, bias and lora_b are sharded 512-wide per core; lora_a and indices
are replicated. The per-token LoRA shrink (s = A @ x) is sharded over
tokens (256/core) and shared via an on-chip AllGather in fp8; the LoRA
expand is folded into the main accumulation as a dense matmul against the
routing-masked shrink (s_masked[t, (l,r)] = (idx[t]==l) * s[t, (l,r)]).

Perf notes (measured on HW):
- The PE streams moving columns at ~1.95 GHz regardless of dtype, so the
  base matmul stays bf16 (fp8 for the base fails the 2e-2 gate: measured
  3.9e-2). The LoRA shrink/expand run as fp8e4 DoubleRow matmuls (2 K-tiles
  per instruction), halving their PE occupancy; their quantization error
  only feeds the small LoRA correction term (total rel err ~6e-3).
- The base matmul is tiled into 8 token groups of 256 and interleaved with
  the shrink chunks at the start so the PE never waits on the big x/w DMA
  streams; shrink inputs ride the Activation-engine DMA queue while
  x/w ride the SP queue.
- Expand groups trail base groups by 2 so the fp8 AllGather hides under
  ~34us of base matmuls; output is stored bf16 (upcast on host).
"""
import json

import numpy as np
import ml_dtypes

import concourse.bass as bass
import concourse.mybir as mybir
import concourse.tile as tile
from concourse.bass_utils import run_bass_kernel_spmd

T, H, O, L, R = 2048, 4096, 4096, 32, 16
N_CORES = 8
O_SH = O // N_CORES          # 512  output cols per core
T_LOC = T // N_CORES         # 256  tokens whose LoRA-shrink this core computes
KB = H // 128                # 32   contraction blocks
LR = L * R                   # 512  stacked (lora, rank) rows
G = 8                        # base-matmul token groups
TG = T // G                  # 256  tokens per group
BF16 = mybir.dt.bfloat16
F32 = mybir.dt.float32
F8 = mybir.dt.float8e4
DR = mybir.MatmulPerfMode.DoubleRow
A_SCALE = 16.0               # lora_a is pre-scaled x16 into fp8
B_SCALE = 64.0               # lora_b is pre-scaled x64 into fp8
D_SCALE = 1.0 / (A_SCALE * B_SCALE)   # undo both during the final combine


def _split_waits(raw: bytes) -> bytes:
    """This walrus build rejects instructions carrying more than one sync
    wait ("Too many sync wait commands"), but Tile attaches one wait per
    producing proc. Hoist all but one wait of each instruction onto
    single-wait NoOps inserted just before it on the same engine — the
    engine executes its stream in order, so the gating is identical."""
    m = json.loads(raw)
    ctr = 0
    for f in m["functions"]:
        for b in f["blocks"]:
            out = []
            for inst in b["instructions"]:
                si = inst.get("sync_info")
                waits = si.get("on_wait") if si else None
                if waits and len(waits) > 1:
                    for w in waits[:-1]:
                        ctr += 1
                        out.append({
                            "debug": inst.get("debug", 0),
                            "engine": inst["engine"],
                            "ins": [],
                            "name": f"I-wsplit-{ctr}",
                            "opcode": "NoOp",
                            "outs": [],
                            "sync_info": {"on_update": [], "on_wait": [w]},
                        })
                    si["on_wait"] = [waits[-1]]
                out.append(inst)
            b["instructions"] = out
    return json.dumps(m).encode()


class _WaitSplitBass(bass.Bass):
    def to_json_bytes(self) -> bytes:
        return _split_waits(super().to_json_bytes())


def _build() -> bass.Bass:
    nc = _WaitSplitBass()
    # all streamed inputs are PE-tile-major: [128 h-partitions, ..., free]
    xG = nc.dram_tensor("xG", [128, G, KB, TG], BF16, kind="ExternalInput")
    xl_r = nc.dram_tensor("xl_r", [128, KB, T_LOC], F8, kind="ExternalInput")
    wTr = nc.dram_tensor("wTr", [128, KB, O_SH], BF16, kind="ExternalInput")
    aTr = nc.dram_tensor("aTr", [128, KB, LR], F8, kind="ExternalInput")
    bTr = nc.dram_tensor("bTr", [128, 4, O_SH], F8, kind="ExternalInput")
    bias_row = nc.dram_tensor("bias_row", [1, O_SH], BF16, kind="ExternalInput")
    idx_bc_d = nc.dram_tensor("idx_bc", [128, T_LOC], F32, kind="ExternalInput")
    lrow_d = nc.dram_tensor("lrow", [128, 4], F32, kind="ExternalInput")
    out = nc.dram_tensor("out", [T, O_SH], BF16, kind="ExternalOutput")

    with tile.TileContext(nc) as tc:
        with (
            tc.tile_pool(name="res", bufs=1) as res,          # long-lived SBUF
            tc.tile_pool(name="stream", bufs=4) as stream,    # streamed SBUF
            tc.tile_pool(name="ps", bufs=2, space="PSUM") as ps,
            tc.tile_pool(name="dram", bufs=1, space="DRAM") as dram,
        ):
            # ------------- shrink-path inputs on the Activation DMA queue ---
            xl_all = res.tile([128, KB, T_LOC], F8, name="xl_all")
            at_all = res.tile([128, KB, LR], F8, name="at_all")
            # first chunk small so the shrink starts as early as possible
            for lo, hi in ((0, 4), (4, 8), (8, 16), (16, 24), (24, 32)):
                nc.scalar.dma_start(xl_all[:, lo:hi, :], xl_r[:, lo:hi, :])
                nc.scalar.dma_start(at_all[:, lo:hi, :], aTr[:, lo:hi, :])
            bias_r = res.tile([1, O_SH], BF16, name="bias_r")
            nc.scalar.dma_start(bias_r[:], bias_row[:])
            idx_bc = res.tile([128, T_LOC], F32, name="idx_bc_t")
            nc.scalar.dma_start(idx_bc[:], idx_bc_d[:])
            lrow = res.tile([128, 4], F32, name="lrow_t")
            nc.scalar.dma_start(lrow[:], lrow_d[:])
            bt_all = res.tile([128, 4, O_SH], F8, name="bt_all")
            nc.gpsimd.dma_start(bt_all[:], bTr[:])

            # ------------- base-path inputs on the SP DMA queue -------------
            # Only the first xs/wt chunks go ahead of the guard below: the SP
            # bulk stream otherwise out-competes the Activation-queue shrink
            # inputs for HBM bandwidth and stalls the AllGather chain.
            wt_all = res.tile([128, KB, O_SH], BF16, name="wt_all")
            xs0 = stream.tile([128, KB, TG], BF16, name="xs", tag="xs", bufs=3)
            nc.sync.dma_start(xs0[:, 0:16, :], xG[:, 0:1, 0:16, :])
            nc.sync.dma_start(wt_all[:, 0:8, :], wTr[:, 0:8, :])
            guard = res.tile([1, 1], F8, name="guard")
            nc.sync.dma_start(guard[:], at_all[:1, KB - 1:KB, LR - 1:LR])
            nc.sync.dma_start(wt_all[:, 8:16, :], wTr[:, 8:16, :])
            nc.sync.dma_start(xs0[:, 16:32, :], xG[:, 0:1, 16:32, :])
            nc.sync.dma_start(wt_all[:, 16:24, :], wTr[:, 16:24, :])
            nc.sync.dma_start(wt_all[:, 24:32, :], wTr[:, 24:32, :])

            ones_t = res.tile([1, 128], BF16, name="ones_t")
            nc.vector.memset(ones_t[:], 1.0)

            # base accumulations land in SBUF (with bias) as each group
            # finishes; the expand is combined during the store.
            base_sb = res.tile([128, 2 * G * O_SH], F32, name="base_sb")

            ps_s = [ps.tile([128, T_LOC], F32, name=f"ps_s{m}", tag=tg)
                    for m, tg in enumerate(["pso0", "pso1", "psd0", "psd1"])]

            # Preamble: interleave the fp8 DoubleRow shrink (each matmul
            # contracts the kb pair (2k2, 2k2+1)) with base group 0 so the PE
            # tracks both DMA streams and the DR matmuls run at full p-state.
            ps_o0 = [ps.tile([128, O_SH], F32, name=f"ps_o0_{t}", tag=f"pso{t}")
                     for t in range(2)]
            for c in range(4):
                for k2 in range(4 * c, 4 * (c + 1)):
                    for m in range(4):
                        nc.tensor.matmul(
                            ps_s[m][:],
                            at_all[:, 2 * k2:2 * k2 + 2, m * 128:(m + 1) * 128],
                            xl_all[:, 2 * k2:2 * k2 + 2, :],
                            start=(k2 == 0),
                            stop=(k2 == 15),
                            perf_mode=DR,
                        )
                for kb in range(8 * c, 8 * (c + 1)):
                    for tt in range(2):
                        nc.tensor.matmul(
                            ps_o0[tt][:],
                            xs0[:, kb, tt * 128:(tt + 1) * 128],
                            wt_all[:, kb, :],
                            start=(kb == 0),
                            stop=(kb == KB - 1),
                        )

            def base_group(g, ps_o):
                xs = stream.tile([128, KB, TG], BF16, name="xs", tag="xs", bufs=3)
                nc.sync.dma_start(xs[:], xG[:, g:g + 1, :, :])
                for kb in range(KB):
                    for tt in range(2):
                        nc.tensor.matmul(
                            ps_o[tt][:],
                            xs[:, kb, tt * 128:(tt + 1) * 128],
                            wt_all[:, kb, :],
                            start=(kb == 0),
                            stop=(kb == KB - 1),
                        )
                for tt in range(2):
                    nc.vector.tensor_tensor(
                        base_sb[:, (2 * g + tt) * O_SH:(2 * g + tt + 1) * O_SH],
                        ps_o[tt][:],
                        bias_bc[:],
                        op=mybir.AluOpType.add,
                    )

            # routing mask + rescale + fp8 downcast, fused:
            # sm = (idx==l(p)) * (A_SCALE * s)
            sm = res.tile([128, 4 * T_LOC], F8, name="sm")
            for m in range(4):
                nc.vector.scalar_tensor_tensor(
                    sm[:, m * T_LOC:(m + 1) * T_LOC],
                    idx_bc[:],
                    lrow[:, m:m + 1],
                    ps_s[m][:],
                    op0=mybir.AluOpType.is_equal,
                    op1=mybir.AluOpType.mult,
                )
            cc_in = dram.tile([LR, T_LOC], F8, name="cc_in")
            nc.gpsimd.dma_start(
                cc_in[:].rearrange("(m p) t -> p m t", p=128),
                sm[:].rearrange("p (m t) -> p m t", t=T_LOC),
            )
            cc_out = dram.tile([N_CORES, LR, T_LOC], F8, name="cc_out",
                               addr_space="Shared")
            nc.gpsimd.collective_compute(
                "AllGather",
                mybir.AluOpType.bypass,
                replica_groups=[list(range(N_CORES))],
                ins=[cc_in.opt()],
                outs=[cc_out.opt()],
            )

            # bias broadcast [128, 512] f32 via K=1 ones-matmul, off the
            # critical path (only needed by group 0's psum copyout)
            bias_ps = ps.tile([128, O_SH], F32, name="bias_ps", tag="psd0")
            nc.tensor.matmul(bias_ps[:], ones_t[:], bias_r[:], start=True, stop=True)
            bias_bc = res.tile([128, O_SH], F32, name="bias_bc")
            nc.vector.tensor_copy(bias_bc[:], bias_ps[:])
            for tt in range(2):
                nc.vector.tensor_tensor(
                    base_sb[:, tt * O_SH:(tt + 1) * O_SH],
                    ps_o0[tt][:],
                    bias_bc[:],
                    op=mybir.AluOpType.add,
                )

            def st_load(g):
                st = stream.tile([128, 4, TG], F8, name="st", tag="st", bufs=3)
                for blk in range(4):
                    # scalar queue: idle after the preamble, and unlike the
                    # gpsimd queue it is not blocked behind the collective
                    nc.scalar.dma_start(
                        st[:, blk:blk + 1, :],
                        cc_out[g, blk * 128:(blk + 1) * 128, :],
                    )
                return st

            def tail(g, st):
                ps_d = [
                    ps.tile([128, O_SH], F32, name=f"ps_d{g}_{t}", tag=f"psd{t}")
                    for t in range(2)
                ]
                for tt in range(2):
                    for d2 in range(2):
                        nc.tensor.matmul(
                            ps_d[tt][:],
                            st[:, 2 * d2:2 * d2 + 2, tt * 128:(tt + 1) * 128],
                            bt_all[:, 2 * d2:2 * d2 + 2, :],
                            start=(d2 == 0),
                            stop=(d2 == 1),
                            perf_mode=DR,
                        )
                ot = stream.tile([128, 2 * O_SH], BF16, name="ot", tag="ot", bufs=2)
                for tt in range(2):
                    nc.vector.scalar_tensor_tensor(
                        ot[:, tt * O_SH:(tt + 1) * O_SH],
                        ps_d[tt][:],
                        D_SCALE,
                        base_sb[:, (2 * g + tt) * O_SH:(2 * g + tt + 1) * O_SH],
                        op0=mybir.AluOpType.mult,
                        op1=mybir.AluOpType.add,
                    )
                dst = out[g * TG:(g + 1) * TG, :].rearrange(
                    "(tt p) o -> p tt o", p=128
                )
                nc.sync.dma_start(dst, ot[:].rearrange("p (tt o) -> p tt o", o=O_SH))

            # expand trails base by four groups: the fp8 AllGather (done
            # ~100us in) is always ready before the first expand matmul;
            # st tiles prefetch one group before their tail consumes them
            sts = {}
            for g in range(1, G):
                ps_o = [ps.tile([128, O_SH], F32, name=f"ps_o{g}_{t}",
                                tag=f"pso{t}") for t in range(2)]
                base_group(g, ps_o)
                if g >= 3:
                    sts[g - 3] = st_load(g - 3)
                if g >= 4:
                    tail(g - 4, sts.pop(g - 4))
            for g in range(G - 4, G):
                if g + 1 < G:
                    sts[g + 1] = st_load(g + 1)
                tail(g, sts.pop(g))
    return nc


_NC_CACHE = None


def build_in_maps(x, weight, bias, lora_a, lora_b, indices):
    bf = ml_dtypes.bfloat16
    f8 = mybir.dt.np(F8)

    # [128 h-partitions, group, kb, token] PE-tile-major layout
    xG = np.ascontiguousarray(
        x.astype(bf).reshape(G, TG, KB, 128).transpose(3, 0, 2, 1))
    aTr = np.ascontiguousarray(
        (lora_a * A_SCALE).astype(f8).reshape(LR, H).T
        .reshape(KB, 128, LR).transpose(1, 0, 2))                   # (128,KB,LR)
    idx_f = indices.astype(np.float32)                              # (T,)
    lrow = np.broadcast_to(
        (np.arange(128)[:, None] // 16).astype(np.float32), (128, 4)
    ).copy()
    lrow = lrow + (np.arange(4)[None, :] * 8).astype(np.float32)    # (128, 4)

    in_maps = []
    for c in range(N_CORES):
        wTc = np.ascontiguousarray(
            weight[c * O_SH:(c + 1) * O_SH, :].astype(bf).T
            .reshape(KB, 128, O_SH).transpose(1, 0, 2))             # (128,KB,O_SH)
        bTc = np.ascontiguousarray(
            (lora_b[:, c * O_SH:(c + 1) * O_SH, :] * B_SCALE).astype(f8)
            .transpose(0, 2, 1).reshape(LR, O_SH)                   # ((l,r), o)
            .reshape(4, 128, O_SH).transpose(1, 0, 2))              # (128,4,O_SH)
        bias_c = np.ascontiguousarray(
            bias[c * O_SH:(c + 1) * O_SH].astype(bf))[None, :]
        idx_bc = np.broadcast_to(
            idx_f[c * T_LOC:(c + 1) * T_LOC][None, :], (128, T_LOC)
        ).copy()
        xl_c = np.ascontiguousarray(
            x[c * T_LOC:(c + 1) * T_LOC, :].astype(f8).T
            .reshape(KB, 128, T_LOC).transpose(1, 0, 2))            # (128,KB,T_LOC)
        in_maps.append({
            "xG": xG, "xl_r": xl_c, "wTr": wTc, "aTr": aTr, "bTr": bTc,
            "bias_row": bias_c, "idx_bc": idx_bc, "lrow": lrow,
        })
    return in_maps


def kernel(x, weight, bias, lora_a, lora_b, indices):
    global _NC_CACHE
    in_maps = build_in_maps(x, weight, bias, lora_a, lora_b, indices)
    if _NC_CACHE is None:
        _NC_CACHE = _build()
    r = run_bass_kernel_spmd(_NC_CACHE, in_maps, core_ids=list(range(N_CORES)))
    return np.concatenate(
        [r.results[c]["out"].astype(np.float32) for c in range(N_CORES)], axis=1)


# revision 13
# speedup vs baseline: 1.0990x; 1.0029x over previous
"""ColumnParallelLinear + per-token LoRA (punica add_lora) on 8 NeuronCores.

out = x @ W^T + b + B[idx] @ (A[idx] @ x^T), idx==-1 skips LoRA.

Sharding: tensor-parallel over the output dim (vLLM ColumnParallelLinear):
weight, bias and lora_b are sharded 512-wide per core; lora_a and indices
are replicated. The per-token LoRA shrink (s = A @ x) is sharded over
tokens (256/core) and shared via an on-chip AllGather in fp8; the LoRA
expand is folded into the main accumulation as a dense matmul against the
routing-masked shrink (s_masked[t, (l,r)] = (idx[t]==l) * s[t, (l,r)]).

Perf notes (measured on HW):
- The PE streams moving columns at ~1.95 GHz regardless of dtype, so the
  base matmul stays bf16 (fp8 for the base fails the 2e-2 gate: measured
  3.9e-2). The LoRA shrink/expand run as fp8e4 DoubleRow matmuls (2 K-tiles
  per instruction), halving their PE occupancy; their quantization error
  only feeds the small LoRA correction term (total rel err ~6e-3).
- The base matmul is tiled into 8 token groups of 256 and interleaved with
  the shrink chunks at the start so the PE never waits on the big x/w DMA
  streams; shrink inputs ride the Activation-engine DMA queue while
  x/w ride the SP queue.
- Expand groups trail base groups by 2 so the fp8 AllGather hides under
  ~34us of base matmuls; output is stored bf16 (upcast on host).
"""
import json

import numpy as np
import ml_dtypes

import concourse.bass as bass
import concourse.mybir as mybir
import concourse.tile as tile
from concourse.bass_utils import run_bass_kernel_spmd

T, H, O, L, R = 2048, 4096, 4096, 32, 16
N_CORES = 8
O_SH = O // N_CORES          # 512  output cols per core
T_LOC = T // N_CORES         # 256  tokens whose LoRA-shrink this core computes
KB = H // 128                # 32   contraction blocks
LR = L * R                   # 512  stacked (lora, rank) rows
G = 8                        # base-matmul token groups
TG = T // G                  # 256  tokens per group
BF16 = mybir.dt.bfloat16
F32 = mybir.dt.float32
F8 = mybir.dt.float8e4
DR = mybir.MatmulPerfMode.DoubleRow
A_SCALE = 16.0               # lora_a is pre-scaled x16 into fp8
B_SCALE = 64.0               # lora_b is pre-scaled x64 into fp8
D_SCALE = 1.0 / (A_SCALE * B_SCALE)   # undo both during the final combine


def _split_waits(raw: bytes) -> bytes:
    """This walrus build rejects instructions carrying more than one sync
    wait ("Too many sync wait commands"), but Tile attaches one wait per
    producing proc. Hoist all but one wait of each instruction onto
    single-wait NoOps inserted just before it on the same engine — the
    engine executes its stream in order, so the gating is identical."""
    m = json.loads(raw)
    ctr = 0
    for f in m["functions"]:
        for b in f["blocks"]:
            out = []
            for inst in b["instructions"]:
                si = inst.get("sync_info")
                waits = si.get("on_wait") if si else None
                if waits and len(waits) > 1:
                    for w in waits[:-1]:
                        ctr += 1
                        out.append({
                            "debug": inst.get("debug", 0),
                            "engine": inst["engine"],
                            "ins": [],
                            "name": f"I-wsplit-{ctr}",
                            "opcode": "NoOp",
                            "outs": [],
                            "sync_info": {"on_update": [], "on_wait": [w]},
                        })
                    si["on_wait"] = [waits[-1]]
                out.append(inst)
            b["instructions"] = out
    return json.dumps(m).encode()


class _WaitSplitBass(bass.Bass):
    def to_json_bytes(self) -> bytes:
        return _split_waits(super().to_json_bytes())


def _build() -> bass.Bass:
    nc = _WaitSplitBass()
    # all streamed inputs are PE-tile-major: [128 h-partitions, ..., free]
    xG = nc.dram_tensor("xG", [128, G, KB, TG], BF16, kind="ExternalInput")
    xl_r = nc.dram_tensor("xl_r", [128, KB, T_LOC], F8, kind="ExternalInput")
    wTr = nc.dram_tensor("wTr", [128, KB, O_SH], BF16, kind="ExternalInput")
    aTr = nc.dram_tensor("aTr", [128, KB, LR], F8, kind="ExternalInput")
    bTr = nc.dram_tensor("bTr", [128, 4, O_SH], F8, kind="ExternalInput")
    bias_row = nc.dram_tensor("bias_row", [1, O_SH], BF16, kind="ExternalInput")
    idx_bc_d = nc.dram_tensor("idx_bc", [128, T_LOC], F32, kind="ExternalInput")
    lrow_d = nc.dram_tensor("lrow", [128, 4], F32, kind="ExternalInput")
    out = nc.dram_tensor("out", [T, O_SH], BF16, kind="ExternalOutput")

    with tile.TileContext(nc) as tc:
        with (
            tc.tile_pool(name="res", bufs=1) as res,          # long-lived SBUF
            tc.tile_pool(name="stream", bufs=4) as stream,    # streamed SBUF
            tc.tile_pool(name="ps", bufs=2, space="PSUM") as ps,
            tc.tile_pool(name="dram", bufs=1, space="DRAM") as dram,
        ):
            # dummy collective first: absorbs the collective engine's one-time
            # setup cost so the real AllGather starts promptly
            ccw_in = dram.tile([1, 4], F8, name="ccw_in")
            ccw_out = dram.tile([N_CORES, 1, 4], F8, name="ccw_out",
                                addr_space="Shared")
            nc.gpsimd.collective_compute(
                "AllGather",
                mybir.AluOpType.bypass,
                replica_groups=[list(range(N_CORES))],
                ins=[ccw_in.opt()],
                outs=[ccw_out.opt()],
            )
            # ------------- shrink-path inputs on the Activation queue -------
            xl_all = res.tile([128, KB, T_LOC], F8, name="xl_all")
            at_all = res.tile([128, KB, LR], F8, name="at_all")
            # first chunk small so the shrink starts as early as possible
            for lo, hi in ((0, 4), (4, 8), (8, 16), (16, 24), (24, 32)):
                nc.scalar.dma_start(xl_all[:, lo:hi, :], xl_r[:, lo:hi, :])
                nc.scalar.dma_start(at_all[:, lo:hi, :], aTr[:, lo:hi, :])
            bias_r = res.tile([1, O_SH], BF16, name="bias_r")
            nc.scalar.dma_start(bias_r[:], bias_row[:])
            idx_bc = res.tile([128, T_LOC], F32, name="idx_bc_t")
            nc.scalar.dma_start(idx_bc[:], idx_bc_d[:])
            lrow = res.tile([128, 4], F32, name="lrow_t")
            nc.scalar.dma_start(lrow[:], lrow_d[:])
            bt_all = res.tile([128, 4, O_SH], F8, name="bt_all")
            nc.scalar.dma_start(bt_all[:], bTr[:])

            # ------------- base-path inputs on the SP DMA queue -------------
            # Group 0's x and the full weight load run ahead of the guard;
            # the rest of the x stream waits for the shrink inputs to land,
            # since the SP bulk stream otherwise out-competes them for HBM
            # bandwidth and stalls the AllGather chain.
            wt_all = res.tile([128, KB, O_SH], BF16, name="wt_all")
            xs0 = stream.tile([128, KB, TG], BF16, name="xs", tag="xs", bufs=3)
            nc.sync.dma_start(xs0[:, 0:16, :], xG[:, 0:1, 0:16, :])
            nc.sync.dma_start(wt_all[:, 0:8, :], wTr[:, 0:8, :])
            nc.sync.dma_start(wt_all[:, 8:16, :], wTr[:, 8:16, :])
            nc.sync.dma_start(xs0[:, 16:32, :], xG[:, 0:1, 16:32, :])
            nc.sync.dma_start(wt_all[:, 16:24, :], wTr[:, 16:24, :])
            nc.sync.dma_start(wt_all[:, 24:32, :], wTr[:, 24:32, :])
            guard = res.tile([1, 2], F8, name="guard")
            nc.sync.dma_start(guard[:, 0:1], at_all[:1, KB - 1:KB, LR - 1:LR])
            nc.sync.dma_start(guard[:, 1:2], xl_all[:1, KB - 1:KB, T_LOC - 1:T_LOC])

            ones_t = res.tile([1, 128], BF16, name="ones_t")
            nc.vector.memset(ones_t[:], 1.0)

            # base accumulations land in SBUF (with bias) as each group
            # finishes; the expand is combined during the store.
            base_sb = res.tile([128, 2 * G * O_SH], F32, name="base_sb")

            ps_s = [ps.tile([128, T_LOC], F32, name=f"ps_s{m}", tag=tg)
                    for m, tg in enumerate(["pso0", "pso1", "psd0", "psd1"])]

            # Preamble: interleave the fp8 DoubleRow shrink (each matmul
            # contracts the kb pair (2k2, 2k2+1)) with base group 0 so the PE
            # tracks both DMA streams and the DR matmuls run at full p-state.
            ps_o0 = [ps.tile([128, O_SH], F32, name=f"ps_o0_{t}", tag=f"pso{t}")
                     for t in range(2)]
            for c in range(4):
                for k2 in range(4 * c, 4 * (c + 1)):
                    for m in range(4):
                        nc.tensor.matmul(
                            ps_s[m][:],
                            at_all[:, 2 * k2:2 * k2 + 2, m * 128:(m + 1) * 128],
                            xl_all[:, 2 * k2:2 * k2 + 2, :],
                            start=(k2 == 0),
                            stop=(k2 == 15),
                            perf_mode=DR,
                        )
                for kb in range(8 * c, 8 * (c + 1)):
                    for tt in range(2):
                        nc.tensor.matmul(
                            ps_o0[tt][:],
                            xs0[:, kb, tt * 128:(tt + 1) * 128],
                            wt_all[:, kb, :],
                            start=(kb == 0),
                            stop=(kb == KB - 1),
                        )

            def base_group(g, ps_o):
                xs = stream.tile([128, KB, TG], BF16, name="xs", tag="xs", bufs=3)
                nc.sync.dma_start(xs[:], xG[:, g:g + 1, :, :])
                for kb in range(KB):
                    for tt in range(2):
                        nc.tensor.matmul(
                            ps_o[tt][:],
                            xs[:, kb, tt * 128:(tt + 1) * 128],
                            wt_all[:, kb, :],
                            start=(kb == 0),
                            stop=(kb == KB - 1),
                        )
                for tt in range(2):
                    nc.vector.tensor_tensor(
                        base_sb[:, (2 * g + tt) * O_SH:(2 * g + tt + 1) * O_SH],
                        ps_o[tt][:],
                        bias_bc[:],
                        op=mybir.AluOpType.add,
                    )

            # routing mask + rescale + fp8 downcast, fused:
            # sm = (idx==l(p)) * (A_SCALE * s)
            sm = res.tile([128, 4 * T_LOC], F8, name="sm")
            for m in range(4):
                nc.vector.scalar_tensor_tensor(
                    sm[:, m * T_LOC:(m + 1) * T_LOC],
                    idx_bc[:],
                    lrow[:, m:m + 1],
                    ps_s[m][:],
                    op0=mybir.AluOpType.is_equal,
                    op1=mybir.AluOpType.mult,
                )
            cc_in = dram.tile([LR, T_LOC], F8, name="cc_in")
            nc.gpsimd.dma_start(
                cc_in[:].rearrange("(m p) t -> p m t", p=128),
                sm[:].rearrange("p (m t) -> p m t", t=T_LOC),
            )
            cc_out = dram.tile([N_CORES, LR, T_LOC], F8, name="cc_out",
                               addr_space="Shared")
            nc.gpsimd.collective_compute(
                "AllGather",
                mybir.AluOpType.bypass,
                replica_groups=[list(range(N_CORES))],
                ins=[cc_in.opt()],
                outs=[cc_out.opt()],
            )

            # bias broadcast [128, 512] f32 via K=1 ones-matmul, off the
            # critical path (only needed by group 0's psum copyout)
            bias_ps = ps.tile([128, O_SH], F32, name="bias_ps", tag="psd0")
            nc.tensor.matmul(bias_ps[:], ones_t[:], bias_r[:], start=True, stop=True)
            bias_bc = res.tile([128, O_SH], F32, name="bias_bc")
            nc.vector.tensor_copy(bias_bc[:], bias_ps[:])
            for tt in range(2):
                nc.vector.tensor_tensor(
                    base_sb[:, tt * O_SH:(tt + 1) * O_SH],
                    ps_o0[tt][:],
                    bias_bc[:],
                    op=mybir.AluOpType.add,
                )

            def st_load(g):
                st = stream.tile([128, 4, TG], F8, name="st", tag="st", bufs=3)
                for blk in range(4):
                    # scalar queue: idle after the preamble, and unlike the
                    # gpsimd queue it is not blocked behind the collective
                    nc.scalar.dma_start(
                        st[:, blk:blk + 1, :],
                        cc_out[g, blk * 128:(blk + 1) * 128, :],
                    )
                return st

            def tail(g, st):
                ps_d = [
                    ps.tile([128, O_SH], F32, name=f"ps_d{g}_{t}", tag=f"psd{t}")
                    for t in range(2)
                ]
                for tt in range(2):
                    for d2 in range(2):
                        nc.tensor.matmul(
                            ps_d[tt][:],
                            st[:, 2 * d2:2 * d2 + 2, tt * 128:(tt + 1) * 128],
                            bt_all[:, 2 * d2:2 * d2 + 2, :],
                            start=(d2 == 0),
                            stop=(d2 == 1),
                            perf_mode=DR,
                        )
                ot = stream.tile([128, 2 * O_SH], BF16, name="ot", tag="ot", bufs=2)
                for tt in range(2):
                    nc.vector.scalar_tensor_tensor(
                        ot[:, tt * O_SH:(tt + 1) * O_SH],
                        ps_d[tt][:],
                        D_SCALE,
                        base_sb[:, (2 * g + tt) * O_SH:(2 * g + tt + 1) * O_SH],
                        op0=mybir.AluOpType.mult,
                        op1=mybir.AluOpType.add,
                    )
                dst = out[g * TG:(g + 1) * TG, :].rearrange(
                    "(tt p) o -> p tt o", p=128
                )
                nc.sync.dma_start(dst, ot[:].rearrange("p (tt o) -> p tt o", o=O_SH))

            # expand trails base by four groups: the fp8 AllGather (done
            # ~100us in) is always ready before the first expand matmul;
            # st tiles prefetch one group before their tail consumes them
            sts = {}
            for g in range(1, G):
                ps_o = [ps.tile([128, O_SH], F32, name=f"ps_o{g}_{t}",
                                tag=f"pso{t}") for t in range(2)]
                base_group(g, ps_o)
                if g >= 3:
                    sts[g - 3] = st_load(g - 3)
                if g >= 4:
                    tail(g - 4, sts.pop(g - 4))
            for g in range(G - 4, G):
                if g + 1 < G:
                    sts[g + 1] = st_load(g + 1)
                tail(g, sts.pop(g))
    return nc


_NC_CACHE = None


def build_in_maps(x, weight, bias, lora_a, lora_b, indices):
    bf = ml_dtypes.bfloat16
    f8 = mybir.dt.np(F8)

    # [128 h-partitions, group, kb, token] PE-tile-major layout
    xG = np.ascontiguousarray(
        x.astype(bf).reshape(G, TG, KB, 128).transpose(3, 0, 2, 1))
    aTr = np.ascontiguousarray(
        (lora_a * A_SCALE).astype(f8).reshape(LR, H).T
        .reshape(KB, 128, LR).transpose(1, 0, 2))                   # (128,KB,LR)
    idx_f = indices.astype(np.float32)                              # (T,)
    lrow = np.broadcast_to(
        (np.arange(128)[:, None] // 16).astype(np.float32), (128, 4)
    ).copy()
    lrow = lrow + (np.arange(4)[None, :] * 8).astype(np.float32)    # (128, 4)

    in_maps = []
    for c in range(N_CORES):
        wTc = np.ascontiguousarray(
            weight[c * O_SH:(c + 1) * O_SH, :].astype(bf).T
            .reshape(KB, 128, O_SH).transpose(1, 0, 2))             # (128,KB,O_SH)
        bTc = np.ascontiguousarray(
            (lora_b[:, c * O_SH:(c + 1) * O_SH, :] * B_SCALE).astype(f8)
            .transpose(0, 2, 1).reshape(LR, O_SH)                   # ((l,r), o)
            .reshape(4, 128, O_SH).transpose(1, 0, 2))              # (128,4,O_SH)
        bias_c = np.ascontiguousarray(
            bias[c * O_SH:(c + 1) * O_SH].astype(bf))[None, :]
        idx_bc = np.broadcast_to(
            idx_f[c * T_LOC:(c + 1) * T_LOC][None, :], (128, T_LOC)
        ).copy()
        xl_c = np.ascontiguousarray(
            x[c * T_LOC:(c + 1) * T_LOC, :].astype(f8).T
            .reshape(KB, 128, T_LOC).transpose(1, 0, 2))            # (128,KB,T_LOC)
        in_maps.append({
            "xG": xG, "xl_r": xl_c, "wTr": wTc, "aTr": aTr, "bTr": bTc,
            "bias_row": bias_c, "idx_bc": idx_bc, "lrow": lrow,
        })
    return in_maps


def kernel(x, weight, bias, lora_a, lora_b, indices):
    global _NC_CACHE
    in_maps = build_in_maps(x, weight, bias, lora_a, lora_b, indices)
    if _NC_CACHE is None:
        _NC_CACHE = _build()
    r = run_bass_kernel_spmd(_NC_CACHE, in_maps, core_ids=list(range(N_CORES)))
    return np.concatenate(
        [r.results[c]["out"].astype(np.float32) for c in range(N_CORES)], axis=1)
